# revision 11
# baseline (speedup 1.0000x reference)
"""Trainium2 Bass kernel for nn_DeformationCorrector.

Math (per particle, F = [[a,b],[c,d]], det F > 0 for this data):
  closed-form 2x2 SVD:  y1 = (a+d)^2 + (c-b)^2,  y2 = (a-d)^2 + (c+b)^2
    sq1 = sqrt(y1), sq2 = sqrt(y2);  sigma1 = (sq1+sq2)/2, sigma2 = (sq1-sq2)/2
  polar rotation R = U@Vh = [[p,-q],[q,p]],  p = (a+d)/sq1, q = (c-b)/sq1
  features (dedup; -1 shifts folded into b1):
    [sq1+sq2, sq1-sq2, a^2+c^2, ab+cd, b^2+d^2, ad-bc]  @ W1eff + b1eff
  MLP 6->128->128->3 (symmetrized W3), then delta = R @ x_sym, out = delta + F.

Distribution: pure data parallel over 8 cores, contiguous shards, weights
replicated. Layout conversions (particle-major elementwise <-> feature-major
matmul) go through cheap DRAM round trips instead of on-chip transposes.

Stage-2 schedule (per 2048-particle unit u = 4 groups x T):
  PE:  [L2(u-1) x4] [L1(u) 4-row-pack] [L3(u-1) 4-col-pack]
  ACT/DVE: relu1 as ONE [128,2048] instruction (alternating engines),
  relu2 as 4x[128,512] split across engines, drain alternating.
  z1 is a single 4-bank PSUM tile so the whole L1 pack releases on one
  event; L3 gets explicit dep edges to all four relu2s so the col-pack
  issues contiguously. Stage-1/3 elementwise runs on GpSimd for blocks
  that overlap stage-2 (head/tail blocks use the fast engines).
"""

import os
from contextlib import ExitStack

import numpy as np

import concourse.bass as bass
import concourse.bacc as bacc
import concourse.tile as tile
from concourse.tile_rust import add_dep_helper
from concourse import mybir
from concourse.bass_utils import run_bass_kernel_spmd

NCORES = 8
P = 128
T = 512                 # matmul moving free dim (one PSUM bank of fp32)
CB = 512                # particles per partition per block
BLK = P * CB            # 65536 particles per block
NBLK = 2
NPC = NBLK * BLK        # 131072 particles per core (padded)
NTOT = NCORES * NPC     # 1048576
N = 1_000_000
HID = 128

CHUNKS_PER_BLK = BLK // T      # 128
GROUPS_PER_BLK = CHUNKS_PER_BLK // 4   # 32 (4 chunks per group: row/col packing)

FP32 = mybir.dt.float32
F32R = mybir.dt.float32r
BF16 = mybir.dt.bfloat16
AF = mybir.ActivationFunctionType
OP = mybir.AluOpType

# knobs: relu2 on ACT K of 8; stage-1/3 gpsimd offload for overlapped blocks
K_R2_ACT_OF8 = int(os.environ.get("K_R2_ACT_OF8", "4"))
K_S1_GPS = int(os.environ.get("K_S1_GPS", "1"))
K_S3_GPS = int(os.environ.get("K_S3_GPS", "1"))
# PE warmup matmul count; drain on DVE every other unit (ACT/DVE balance)
K_WARM_MM = int(os.environ.get("K_WARM_MM", "20"))
K_DRAIN_DVE_OF2 = int(os.environ.get("K_DRAIN_DVE_OF2", "1"))

_built = {}
_last_results = None


def _r2_on_act(c):
    return (c * K_R2_ACT_OF8) % 8 < K_R2_ACT_OF8


def build_program(nblk=NBLK, cb=CB, dbg=False):
    global NBLK, CB
    NBLK_s, CB_s = NBLK, CB
    NBLK, CB = nblk, cb
    BLK_l = P * cb
    NPC_l = nblk * BLK_l
    try:
        nc = _build_impl(nblk, cb, BLK_l, NPC_l, dbg)
    finally:
        NBLK, CB = NBLK_s, CB_s
    return nc


def _build_impl(NBLK, CB, BLK, NPC, dbg=False):
    assert CB == T, 'g-major DRAM layout requires CB == T'
    CHUNKS_PER_BLK = BLK // T
    GROUPS_PER_BLK = CHUNKS_PER_BLK // 4
    SB = 4                              # units per superblock (featfm/x DMA batch)
    n_super = GROUPS_PER_BLK // SB      # 8
    nc = bacc.Bacc(trn_type="TRN2")

    F_in = nc.dram_tensor("F", [NPC, 4], FP32, kind="ExternalInput")
    W1S_in = nc.dram_tensor("W1S", [P, P], BF16, kind="ExternalInput")
    W2_in = nc.dram_tensor("W2", [P, P], BF16, kind="ExternalInput")
    W3S_in = nc.dram_tensor("W3S", [P, 32], BF16, kind="ExternalInput")
    B1_in = nc.dram_tensor("B1", [P, 1], FP32, kind="ExternalInput")
    B2_in = nc.dram_tensor("B2", [P, 1], FP32, kind="ExternalInput")
    B3S_in = nc.dram_tensor("B3S", [P, 1], FP32, kind="ExternalInput")
    OUT = nc.dram_tensor("OUT", [NPC, 4], FP32, kind="ExternalOutput")

    with tile.TileContext(nc) as tc, ExitStack() as ctx:
        consts = ctx.enter_context(tc.tile_pool(name="consts", bufs=1))
        fblk = ctx.enter_context(tc.tile_pool(name="fblk", bufs=NBLK))
        scr = ctx.enter_context(tc.tile_pool(name="scr", bufs=1))
        featp = ctx.enter_context(tc.tile_pool(name="featp", bufs=NBLK))
        dramp = ctx.enter_context(tc.tile_pool(name="dramp", bufs=NBLK, space="DRAM"))
        fmp = ctx.enter_context(tc.tile_pool(name="fmp", bufs=2))
        hp = ctx.enter_context(tc.tile_pool(name="hp", bufs=2))
        xp = ctx.enter_context(tc.tile_pool(name="xp", bufs=2))
        outp = ctx.enter_context(tc.tile_pool(name="outp", bufs=2))
        psz1 = ctx.enter_context(tc.tile_pool(name="psz1", bufs=1, space="PSUM"))
        psz2 = ctx.enter_context(tc.tile_pool(name="psz2", bufs=3, space="PSUM"))
        psx = ctx.enter_context(tc.tile_pool(name="psx", bufs=1, space="PSUM"))

        # ---- constants ----
        w1s_sb = consts.tile([P, P], BF16)
        nc.sync.dma_start(out=w1s_sb[:], in_=W1S_in[:, :])
        w2_sb = consts.tile([P, P], BF16)
        nc.sync.dma_start(out=w2_sb[:], in_=W2_in[:, :])
        w3s_sb = consts.tile([P, 32], BF16)
        nc.sync.dma_start(out=w3s_sb[:], in_=W3S_in[:, :])
        b1_sb = consts.tile([P, 1], FP32)
        nc.sync.dma_start(out=b1_sb[:], in_=B1_in[:, :])
        b2_sb = consts.tile([P, 1], FP32)
        nc.sync.dma_start(out=b2_sb[:], in_=B2_in[:, :])
        b3s_sb = consts.tile([P, 1], FP32)
        nc.sync.dma_start(out=b3s_sb[:], in_=B3S_in[:, :])

        f_tiles = []
        p_tiles = []
        q_tiles = []
        featd_tiles = []
        xd_tiles = []
        warm_trigger = [None]  # featd store inst that gates the PE warmup chain

        # ============ stage 1: particle-major features (in column halves) ============
        H = CB // 2
        for b in range(NBLK):
            # head block splits its two column-halves across DVE || GpSimd
            # (nothing else runs yet, halves are independent); later blocks
            # run fully on gpsimd, overlapped with stage 2.
            offload = K_S1_GPS and b > 0

            f_sb = fblk.tile([P, 4 * CB], FP32, tag="F", name=f"f_sb{b}")
            F_bv = F_in[:, :].rearrange("(b i g j) k -> b i g (j k)", b=NBLK, i=32, g=4)[b]
            # load in column halves so half-0 feature math starts early
            for hh in range(2):
                for g in range(4):
                    nc.sync.dma_start(
                        out=f_sb[32 * g : 32 * g + 32, hh * 2 * CB : (hh + 1) * 2 * CB],
                        in_=F_bv[:, g, hh * 2 * CB : (hh + 1) * 2 * CB],
                    )
            f_tiles.append(f_sb)
            fr = f_sb.rearrange("p (c k) -> p c k", k=4)
            fr2 = f_sb.rearrange("p (c k2 k) -> p c k2 k", k2=2, k=2)

            feat_sb = featp.tile([P, 6 * CB], FP32, tag="feat", name=f"feat_sb{b}")
            fv = feat_sb.rearrange("p (f c) -> p f c", f=6)
            sq_sb = scr.tile([P, 4 * CB], FP32, tag="sq", name=f"sq_sb{b}")
            sqr = sq_sb.rearrange("p (c k) -> p c k", k=4)
            pp_sb = scr.tile([P, 2 * CB], FP32, tag="pp", name=f"pp_sb{b}")
            ppv = pp_sb.rearrange("p (c k2) -> p c k2", k2=2)
            ad_sb = scr.tile([P, CB], FP32, tag="ad", name=f"ad_sb{b}")
            bc_sb = scr.tile([P, CB], FP32, tag="bc", name=f"bc_sb{b}")
            m_sb = scr.tile([P, CB], FP32, tag="m", name=f"m_sb{b}")
            y1_sb = scr.tile([P, CB], FP32, tag="y1", name=f"y1_sb{b}")
            y2_sb = scr.tile([P, CB], FP32, tag="y2", name=f"y2_sb{b}")
            sq1_sb = scr.tile([P, CB], FP32, tag="sq1", name=f"sq1_sb{b}")
            sq2_sb = scr.tile([P, CB], FP32, tag="sq2", name=f"sq2_sb{b}")
            s_sb = scr.tile([P, CB], FP32, tag="s", name=f"s_sb{b}")
            v_sb = scr.tile([P, CB], FP32, tag="v", name=f"v_sb{b}")
            rinv_sb = scr.tile([P, CB], FP32, tag="rinv", name=f"rinv_sb{b}")
            p_sb = fblk.tile([P, CB], FP32, tag="p", name=f"p_sb{b}")
            q_sb = fblk.tile([P, CB], FP32, tag="q", name=f"q_sb{b}")
            p_tiles.append(p_sb)
            q_tiles.append(q_sb)

            featd = dramp.tile([24, BLK // 4], BF16, tag="featd", name=f"featd{b}")
            featd_tiles.append(featd)

            for h in range(2):
                use_gps = offload or h == 1
                ve = nc.gpsimd if use_gps else nc.vector
                cs = slice(h * H, (h + 1) * H)
                av, bv_, cv, dv = (fr[:, cs, k] for k in range(4))
                ac = fr2[:, cs, :, 0]
                bd = fr2[:, cs, :, 1]
                aa, bb, cc, dd = (sqr[:, cs, k] for k in range(4))
                nc.gpsimd.tensor_tensor(
                    out=sq_sb.rearrange("p (c k) -> p c k", k=4)[:, cs, :],
                    in0=fr[:, cs, :], in1=fr[:, cs, :], op=OP.mult)
                ve.tensor_tensor(out=ppv[:, cs, :], in0=ac, in1=bd, op=OP.mult)
                ve.tensor_tensor(out=fv[:, 3, cs], in0=ppv[:, cs, 0], in1=ppv[:, cs, 1], op=OP.add)
                ve.tensor_tensor(out=ad_sb[:, cs], in0=av, in1=dv, op=OP.mult)
                nc.gpsimd.tensor_tensor(out=bc_sb[:, cs], in0=bv_, in1=cv, op=OP.mult)
                ve.tensor_tensor(out=fv[:, 5, cs], in0=ad_sb[:, cs], in1=bc_sb[:, cs], op=OP.subtract)
                ve.tensor_tensor(out=fv[:, 2, cs], in0=aa, in1=cc, op=OP.add)
                ve.tensor_tensor(out=fv[:, 4, cs], in0=bb, in1=dd, op=OP.add)
                ve.tensor_tensor(out=m_sb[:, cs], in0=fv[:, 2, cs], in1=fv[:, 4, cs], op=OP.add)
                if use_gps:
                    # Pool engine only supports tensor_tensor: y = m -+ 2*f5
                    d2 = ad_sb  # ad is dead after f5; reuse as 2*f5 scratch
                    ve.tensor_tensor(out=d2[:, cs], in0=fv[:, 5, cs], in1=fv[:, 5, cs], op=OP.add)
                    ve.tensor_tensor(out=y1_sb[:, cs], in0=m_sb[:, cs], in1=d2[:, cs], op=OP.add)
                    ve.tensor_tensor(out=y2_sb[:, cs], in0=m_sb[:, cs], in1=d2[:, cs], op=OP.subtract)
                    # max not supported on Pool; tiny clamp stays on DVE
                    nc.vector.tensor_scalar(
                        out=y2_sb[:, cs], in0=y2_sb[:, cs], scalar1=0.0, scalar2=None, op0=OP.max)
                else:
                    ve.scalar_tensor_tensor(
                        out=y1_sb[:, cs], in0=fv[:, 5, cs], scalar=2.0, in1=m_sb[:, cs],
                        op0=OP.mult, op1=OP.add)
                    ve.scalar_tensor_tensor(
                        out=y2_sb[:, cs], in0=fv[:, 5, cs], scalar=-2.0, in1=m_sb[:, cs],
                        op0=OP.mult, op1=OP.add)
                    ve.tensor_scalar(
                        out=y2_sb[:, cs], in0=y2_sb[:, cs], scalar1=0.0, scalar2=None, op0=OP.max)
                nc.scalar.activation(out=sq1_sb[:, cs], in_=y1_sb[:, cs], func=AF.Sqrt)
                nc.scalar.activation(out=sq2_sb[:, cs], in_=y2_sb[:, cs], func=AF.Sqrt)
                ve.tensor_tensor(out=fv[:, 0, cs], in0=sq1_sb[:, cs], in1=sq2_sb[:, cs], op=OP.add)
                ve.tensor_tensor(out=fv[:, 1, cs], in0=sq1_sb[:, cs], in1=sq2_sb[:, cs], op=OP.subtract)
                ve.tensor_tensor(out=s_sb[:, cs], in0=av, in1=dv, op=OP.add)
                ve.tensor_tensor(out=v_sb[:, cs], in0=cv, in1=bv_, op=OP.subtract)
                nc.vector.reciprocal_approx_fast(out=rinv_sb[:, cs], in_=sq1_sb[:, cs])
                ve.tensor_tensor(out=p_sb[:, cs], in0=s_sb[:, cs], in1=rinv_sb[:, cs], op=OP.mult)
                ve.tensor_tensor(out=q_sb[:, cs], in0=v_sb[:, cs], in1=rinv_sb[:, cs], op=OP.mult)
                for g in range(4):
                    st = nc.gpsimd.dma_start(
                        out=featd[6 * g : 6 * g + 6, :].rearrange("f (i j) -> i f j", j=T)[:, :, cs],
                        in_=feat_sb[32 * g : 32 * g + 32, :].rearrange("i (f j) -> i f j", j=T)[:, :, cs],
                    )
                    if b == 0 and h == 1 and g == 0:
                        warm_trigger[0] = st

            xd = dramp.tile([12, BLK // 4], FP32, tag="xd", name=f"xd{b}")
            xd_tiles.append(xd)

        # ============ PE warmup: back-to-back dummy matmuls ============
        # The PE HAM clock gate keeps the array at 1.2 GHz until it sees a
        # ~3.4us window of sustained busy; stage-2's dependency gaps never
        # trigger it, so every matmul runs at cold rate (~604ns vs ~218ns).
        # Burn ~4.5us of back-to-back dummies right before stage-2 starts
        # (gated on an early block-0 h1 featd store so they overlap the tail
        # of stage-1 instead of idling afterwards and re-throttling).
        warm_mms = []
        if K_WARM_MM > 0:
            warm_ps = psx.tile([P, T], FP32, tag="x", name="x_warm")
            for i in range(K_WARM_MM):
                mm = nc.tensor.matmul(
                    out=warm_ps[:, :P], lhsT=w2_sb[:], rhs=w1s_sb[:]
                )
                if i == 0 and warm_trigger[0] is not None:
                    add_dep_helper(mm.ins, warm_trigger[0].ins, reason="warmup gate")
                warm_mms.append(mm)

        # ============ stage 2: feature-major MLP (software-pipelined units) ============
        # unit = 2048 particles: 4 groups x T. PE order per steady-state step:
        #   [L2(prev) x4] [L1(cur) 4-pack] [L3(prev) 4-pack]
        units = [
            (b, s, j)
            for b in range(NBLK)
            for s in range(n_super)
            for j in range(SB)
        ]
        featfm_tiles = {}   # (b, s) -> featfm tile
        xsb_tiles = {}      # (b, s) -> x superblock drain tile
        PREFETCH = 5

        def emit_featfm(k):
            b, s, j = units[k]
            if (b, s) in featfm_tiles:
                return
            featd = featd_tiles[b]
            featfm = fmp.tile([P, SB * T], BF16, tag="featfm", name=f"ffm{b}_{s}")
            for g in range(4):
                nc.sync.dma_start(
                    out=featfm[32 * g : 32 * g + 6, :],
                    in_=featd[6 * g : 6 * g + 6, SB * T * s : SB * T * (s + 1)],
                )
            featfm_tiles[(b, s)] = featfm

        # Steady-state PE FIFO per event "relu1(u-1) done":
        #   [L1(u) 4-pack][L2(u-1) x4][L3(u-2) 4-pack]
        # relu1(u) (split ACT||DVE) then overlaps L2(u-1)+L3(u-2); L3 lags two
        # units so its relu2 deps resolved a full period ago.
        ctx1 = None  # unit u-1: (b, s, j, h1)
        ctx2 = None  # unit u-2: (b, s, j, h2s, r2s)
        cu = 0     # unit counter
        cr2 = 0    # relu2 instruction counter
        last_q = {"pe": None, "act": None, "dve": None}
        if warm_mms:
            last_q["pe"] = warm_mms[-1]
            for a, bm in zip(warm_mms, warm_mms[1:]):
                add_dep_helper(bm.ins, a.ins, reason="warmup chain")

        def chain(qn, inst):
            # pin an engine FIFO to an explicit order: same-engine edges cost
            # nothing at runtime but stop the scheduler from reordering the
            # queue based on its (imperfect) readiness model
            if inst is None:
                return
            if last_q[qn] is not None:
                add_dep_helper(inst.ins, last_q[qn].ins, reason=f"{qn} order chain")
            last_q[qn] = inst

        for idx in range(len(units) + 2):
            # ---- prefetch featfm a few units ahead ----
            for k in range(idx, min(idx + PREFETCH + 1, len(units))):
                emit_featfm(k)

            # ---- L1 + relu1 of current unit ----
            cur = None
            l1_mms = []
            r1a = r1b = None
            if idx < len(units):
                b, s, j = units[idx]
                featfm = featfm_tiles[(b, s)]
                ffm_gv = featfm.rearrange("(g r) c -> g r c", g=4)
                z1 = psz1.tile([P, 4 * T], FP32, tag="z1", name=f"z1_{b}_{s}_{j}")
                for g in range(4):
                    mm = nc.tensor.matmul(
                        out=z1[:, g * T : (g + 1) * T],
                        lhsT=w1s_sb[32 * g : 32 * g + 6, :],
                        rhs=ffm_gv[g, :6, j * T : (j + 1) * T],
                        tile_position=(32 * g, 0),
                    )
                    l1_mms.append(mm)
                # separate tiles per relu1 half: a shared tile would WAW-
                # serialize the two writers at tile granularity
                h1a = hp.tile([P, 2 * T], BF16, tag="h1a", name=f"h1a_{b}_{s}_{j}", bufs=2)
                h1b = hp.tile([P, 2 * T], BF16, tag="h1b", name=f"h1b_{b}_{s}_{j}", bufs=2)
                r1a = nc.scalar.activation(
                    out=h1a[:], in_=z1[:, : 2 * T], func=AF.Relu, bias=b1_sb[:]
                )
                r1b = nc.vector.tensor_scalar(
                    out=h1b[:], in0=z1[:, 2 * T :], scalar1=b1_sb[:],
                    scalar2=0.0, op0=OP.add, op1=OP.max,
                )
                if j == 0:
                    xsb_tiles[(b, s)] = xp.tile(
                        [P, SB * T], FP32, tag="xsb", name=f"xsb{b}_{s}"
                    )
                cur = (b, s, j, (h1a, h1b))
                cu += 1

            # ---- L2 + relu2 of unit u-1 ----
            nxt2 = None
            l2_mms = []
            r2_act = []
            r2_dve = []
            if ctx1 is not None:
                pb, ps, pj, ph1 = ctx1
                ph1a, ph1b = ph1
                r2s = []
                h2s = []
                for g in range(4):
                    z2 = psz2.tile([P, T], FP32, tag="z2", name=f"z2_{pb}_{ps}_{pj}_{g}")
                    rhs_h1 = ph1a if g < 2 else ph1b
                    mm = nc.tensor.matmul(
                        out=z2[:], lhsT=w2_sb[:],
                        rhs=rhs_h1[:, (g % 2) * T : (g % 2 + 1) * T],
                    )
                    l2_mms.append(mm)
                    h2 = hp.tile([P, T], BF16, tag="h2", name=f"h2_{pb}_{ps}_{pj}_{g}", bufs=10)
                    if _r2_on_act(cr2):
                        r2 = nc.scalar.activation(
                            out=h2[:], in_=z2[:], func=AF.Relu, bias=b2_sb[:]
                        )
                        r2_act.append(r2)
                    else:
                        r2 = nc.vector.tensor_scalar(
                            out=h2[:], in0=z2[:], scalar1=b2_sb[:],
                            scalar2=0.0, op0=OP.add, op1=OP.max,
                        )
                        r2_dve.append(r2)
                    cr2 += 1
                    r2s.append(r2)
                    h2s.append(h2)
                nxt2 = (pb, ps, pj, h2s, r2s)

            # ---- L3 + drain of unit u-2 ----
            l3_mms = []
            drain = None
            drain_on_dve = False
            if ctx2 is not None:
                qb, qs, qj, h2s2, r2s2 = ctx2
                x_ps = psx.tile([P, T], FP32, tag="x", name=f"xps{qb}_{qs}_{qj}")
                for g in range(4):
                    l3 = nc.tensor.matmul(
                        out=x_ps[32 * g : 32 * g + 32, :],
                        lhsT=w3s_sb[:, :],
                        rhs=h2s2[g][:],
                        tile_position=(0, 32 * g),
                    )
                    l3_mms.append(l3)
                x_sb = xsb_tiles[(qb, qs)]
                drain_on_dve = K_DRAIN_DVE_OF2 and (cu % 2 == 0)
                if drain_on_dve:
                    drain = nc.vector.tensor_scalar(
                        out=x_sb[:, qj * T : (qj + 1) * T], in0=x_ps[:],
                        scalar1=b3s_sb[:], scalar2=None, op0=OP.add,
                    )
                else:
                    drain = nc.scalar.activation(
                        out=x_sb[:, qj * T : (qj + 1) * T], in_=x_ps[:],
                        func=AF.Identity, bias=b3s_sb[:],
                    )
                if qj == SB - 1:
                    xd = xd_tiles[qb]
                    for g in range(4):
                        nc.sync.dma_start(
                            out=xd[3 * g : 3 * g + 3, SB * T * qs : SB * T * (qs + 1)],
                            in_=x_sb[32 * g : 32 * g + 3, :],
                        )

            # ---- wire the engine FIFOs for this steady-state step ----
            # PE: [L1(u) pack][L2(u-1) x4][L3(u-2) pack]
            for mm in l1_mms + l2_mms + l3_mms:
                chain("pe", mm)
            # ACT: first relu2 of u-1 (frees the wrapped z2 slot for L2-g3),
            # then relu1a(u) (critical for next period), rest, drain(u-2)
            if r2_act:
                chain("act", r2_act[0])
            chain("act", r1a)
            for r2 in r2_act[1:]:
                chain("act", r2)
            if drain is not None and not drain_on_dve:
                chain("act", drain)
            # DVE: relu1b(u) first, then u-1's relu2 share
            chain("dve", r1b)
            for r2 in r2_dve:
                chain("dve", r2)
            if drain is not None and drain_on_dve:
                chain("dve", drain)

            ctx2 = nxt2
            ctx1 = cur

        # ============ stage 3: particle-major backend ============
        for b in range(NBLK):
            # tail block splits column-halves across DVE || GpSimd; earlier
            # blocks run fully on gpsimd (overlapped with stage 2).
            offload = K_S3_GPS and b < NBLK - 1

            xd = xd_tiles[b]
            f_sb = f_tiles[b]
            p_sb = p_tiles[b]
            q_sb = q_tiles[b]
            fr = f_sb.rearrange("p (c k) -> p c k", k=4)

            xs_all = xp.tile([P, 3 * CB], FP32, tag="xsall", name=f"xsall{b}")
            xs_v = xs_all.rearrange("p (k c) -> p k c", k=3)
            for g in range(4):
                nc.sync.dma_start(
                    out=xs_all[32 * g : 32 * g + 32, :].rearrange("i (k j) -> i k j", j=T),
                    in_=xd[3 * g : 3 * g + 3, :].rearrange("k (i j) -> i k j", j=T),
                )
            pall = scr.tile([P, 3 * CB], FP32, tag="pall", name=f"pall{b}")
            pall_v = pall.rearrange("p (k c) -> p k c", k=3)
            qall = scr.tile([P, 3 * CB], FP32, tag="qall", name=f"qall{b}")
            qall_v = qall.rearrange("p (k c) -> p k c", k=3)
            out_sb = outp.tile([P, 4 * CB], FP32, tag="out", name=f"out_sb{b}")
            ov = out_sb.rearrange("p (c k) -> p c k", k=4)
            t0 = scr.tile([P, CB], FP32, tag="t0", name=f"t0_{b}")
            t1 = scr.tile([P, CB], FP32, tag="t1", name=f"t1_{b}")
            t2 = scr.tile([P, CB], FP32, tag="t2", name=f"t2_{b}")
            t3 = scr.tile([P, CB], FP32, tag="t3", name=f"t3_{b}")
            for h in range(2):
                e = nc.gpsimd if (offload or h == 1) else nc.vector
                cs = slice(h * H, (h + 1) * H)
                e.tensor_tensor(
                    out=pall_v[:, :, cs], in0=xs_v[:, :, cs],
                    in1=p_sb[:, cs].unsqueeze(1).to_broadcast([P, 3, H]), op=OP.mult,
                )
                e.tensor_tensor(
                    out=qall_v[:, :, cs], in0=xs_v[:, :, cs],
                    in1=q_sb[:, cs].unsqueeze(1).to_broadcast([P, 3, H]), op=OP.mult,
                )
                e.tensor_tensor(out=t0[:, cs], in0=pall_v[:, 0, cs], in1=qall_v[:, 1, cs], op=OP.subtract)
                e.tensor_tensor(out=ov[:, cs, 0], in0=t0[:, cs], in1=fr[:, cs, 0], op=OP.add)
                e.tensor_tensor(out=t1[:, cs], in0=pall_v[:, 1, cs], in1=qall_v[:, 2, cs], op=OP.subtract)
                e.tensor_tensor(out=ov[:, cs, 1], in0=t1[:, cs], in1=fr[:, cs, 1], op=OP.add)
                e.tensor_tensor(out=t2[:, cs], in0=qall_v[:, 0, cs], in1=pall_v[:, 1, cs], op=OP.add)
                e.tensor_tensor(out=ov[:, cs, 2], in0=t2[:, cs], in1=fr[:, cs, 2], op=OP.add)
                e.tensor_tensor(out=t3[:, cs], in0=qall_v[:, 1, cs], in1=pall_v[:, 2, cs], op=OP.add)
                e.tensor_tensor(out=ov[:, cs, 3], in0=t3[:, cs], in1=fr[:, cs, 3], op=OP.add)

            OUT_bv = OUT[:, :].rearrange("(b i g j) k -> b i g (j k)", b=NBLK, i=32, g=4)[b]
            for g in range(4):
                nc.sync.dma_start(out=OUT_bv[:, g, :], in_=out_sb[32 * g : 32 * g + 32, :])

    nc.finalize()
    return nc


def prep_weights(W1, b1, W2, b2, W3, b3):
    """Host-side weight transforms (tiny)."""
    W1 = np.asarray(W1, np.float32)
    b1 = np.asarray(b1, np.float32)
    W2 = np.asarray(W2, np.float32)
    b2 = np.asarray(b2, np.float32)
    W3 = np.asarray(W3, np.float32)
    b3 = np.asarray(b3, np.float32)
    # features: [sq1+sq2, sq1-sq2, f2, f3, f4, f5]
    W1eff = np.stack(
        [0.5 * W1[0], 0.5 * W1[1], W1[2], W1[3] + W1[4], W1[5], W1[6]], axis=0
    )  # [6, 128]
    b1eff = b1 - (W1[0] + W1[1] + W1[2] + W1[5] + W1[6])
    W1S = np.zeros((P, P), np.float32)
    for g in range(4):
        W1S[32 * g : 32 * g + 6, :] = W1eff
    # symmetrized third layer: x_sym = [x00, (x01+x10)/2, x11]
    W3S = np.zeros((P, 32), np.float32)
    W3S[:, 0] = W3[:, 0]
    W3S[:, 1] = 0.5 * (W3[:, 1] + W3[:, 2])
    W3S[:, 2] = W3[:, 3]
    b3S3 = np.array([b3[0], 0.5 * (b3[1] + b3[2]), b3[3]], np.float32)
    B3S = np.zeros((P, 1), np.float32)
    for j in range(4):
        B3S[32 * j : 32 * j + 3, 0] = b3S3
    import ml_dtypes
    return {
        "W1S": W1S.astype(ml_dtypes.bfloat16),
        "W2": W2.astype(ml_dtypes.bfloat16),
        "W3S": W3S.astype(ml_dtypes.bfloat16),
        "B1": b1eff.reshape(P, 1).astype(np.float32),
        "B2": b2.reshape(P, 1).astype(np.float32),
        "B3S": B3S,
    }


def kernel(F, W1, b1, W2, b2, W3, b3):
    global _last_results
    F = np.asarray(F, np.float32).reshape(-1, 4)
    n = F.shape[0]
    assert n == N, f"expected {N} particles, got {n}"

    if "nc" not in _built:
        _built["nc"] = build_program()
    nc = _built["nc"]

    wmaps = prep_weights(W1, b1, W2, b2, W3, b3)
    Fpad = np.empty((NTOT, 4), np.float32)
    Fpad[:n] = F
    Fpad[n:] = np.array([1.0, 0.1, 0.0, 1.0], np.float32)

    in_maps = []
    for i in range(NCORES):
        m = {"F": np.ascontiguousarray(Fpad[i * NPC : (i + 1) * NPC])}
        m.update(wmaps)
        in_maps.append(m)

    res = run_bass_kernel_spmd(nc, in_maps, core_ids=list(range(NCORES)))
    _last_results = res
    out = np.concatenate([r["OUT"] for r in res.results], axis=0)[:n]
    return out.reshape(n, 2, 2).astype(np.float32)

